# revision 15
# baseline (speedup 1.0000x reference)
"""Trainium2 Bass kernel for a pre-LN transformer block (attention + FFN).

Sharding over 8 NeuronCores: core (b, g) = batch b (0..3) x head-group g (0..1).
Each core runs LN1 + QKV (its 8 heads) + causal attention + its slice of the
output projection for its batch; a pairwise bf16 ReduceScatter (2 chunks,
first overlapped with attention of the second query half) sums the two
head-groups' partial attn_out; each core then finishes 1024 rows
(residual + LN2 + full FFN).

The prep pipeline (LN1 + h^T transposes + q/k/v projections) is FUSED into
the attention qc loop: attention is scalar-engine-bound (the exp stream),
so group g+1's prep matmuls are dripped one-per-step into qc g's attention
steps where they soak up the idle PE time.  Likewise the FFN half-0
residual+LN2+transpose prologue is dripped into late qc==3 (after the first
ReduceScatter has landed), so FF1 can start the moment attention drains.

Attention inner loop: the two heads of a q/k pair live on partitions 0:64 and
64:128, so their K=64 score matmuls run CONCURRENTLY in the PE array via
row-group tiling; both heads' scores land in one 2-bank PSUM tile so a
SINGLE scalar-engine exp (the critical resource) covers both.  Softmax
denominators via a ones-column on V and reciprocal_approx_fast.
"""
import sys

if "/opt/trn_rl_repo" not in sys.path:
    sys.path.insert(0, "/opt/trn_rl_repo")

import contextlib
from collections import deque

import numpy as np
import ml_dtypes

import concourse.bass as bass
import concourse.bacc as bacc
import concourse.tile as tile
from concourse import mybir
from concourse.bass_utils import run_bass_kernel_spmd

F32 = mybir.dt.float32
F32R = mybir.dt.float32r
BF16 = mybir.dt.bfloat16
AF = mybir.ActivationFunctionType
OP = mybir.AluOpType

B, S, D, H = 4, 2048, 1024, 16
HD = D // H
FF = 4 * D
EPS = 1e-5
GH = 8          # heads per core
NP = 128        # partitions
SC = S // NP    # 16 seq chunks of 128
DC = D // NP    # 8 d-chunks
QC = S // 512   # 4 q-chunks of 512
FC = FF // NP   # 32 ff chunks of 128
RROWS = S // 2  # 1024 rows finalized per core


def _ln_stats(nc, pool, x_ap, eps_sb):
    """bn_stats/bn_aggr mean+rstd for a [128, D] fp32 tile. Returns mv tile;
    mv[:,0:1]=mean, mv[:,1:2]=rstd.

    rstd = exp(-0.5*ln(var+eps)): Ln and Exp live in the SAME activation
    table set (natural_log_exp_and_others) as the attention/softmax Exp, so
    LN chunks interleaved into the exp stream cost no ACT_TABLE_LOAD (~2.7us
    per switch) — Sqrt would force a table swap both ways every chunk."""
    stats = pool.tile([NP, 2, 6], F32, tag="lnstats")
    nc.vector.bn_stats(out=stats[:, 0, :], in_=x_ap[:, 0:512])
    nc.vector.bn_stats(out=stats[:, 1, :], in_=x_ap[:, 512:1024])
    mv = pool.tile([NP, 2], F32, tag="lnmv")
    nc.vector.bn_aggr(out=mv[:], in_=stats[:])
    nc.scalar.activation(out=mv[:, 1:2], in_=mv[:, 1:2], func=AF.Ln,
                         bias=eps_sb[:], scale=1.0)
    nc.scalar.activation(out=mv[:, 1:2], in_=mv[:, 1:2], func=AF.Exp,
                         bias=0.0, scale=-0.5)
    return mv


def build(apply_ln1_affine=False, apply_ln2_affine=False,
          add_bproj=False, add_bff1=False, add_bff2=False):
    nc = bacc.Bacc("TRN2", num_devices=8)

    # ---- DRAM parameters (per-core shards, laid out host-side) ----
    x_p = nc.declare_dram_parameter("x", [S, D], F32, isOutput=False)
    xres_p = nc.declare_dram_parameter("x_res", [RROWS, D], F32, isOutput=False)
    wq_p = nc.declare_dram_parameter("wq", [NP, DC, 512], BF16, isOutput=False)
    wk_p = nc.declare_dram_parameter("wk", [NP, DC, 512], BF16, isOutput=False)
    wv_p = nc.declare_dram_parameter("wv", [NP, DC, 512], BF16, isOutput=False)
    wproj_p = nc.declare_dram_parameter("wproj", [NP, 4, D], BF16, isOutput=False)
    wff1_p = nc.declare_dram_parameter("wff1", [FC, NP, DC * NP], BF16, isOutput=False)
    wff2_p = nc.declare_dram_parameter("wff2", [FC, NP, D], BF16, isOutput=False)
    if add_bproj:
        bproj_p = nc.declare_dram_parameter("bproj", [1, D], F32, isOutput=False)
    if add_bff1:
        bff1_p = nc.declare_dram_parameter("bff1", [NP, FC], F32, isOutput=False)
    if add_bff2:
        bff2_p = nc.declare_dram_parameter("bff2", [1, D], F32, isOutput=False)
    ident_p = nc.declare_dram_parameter("ident", [NP, NP], BF16, isOutput=False)
    mask_p = nc.declare_dram_parameter("mask", [NP, NP], BF16, isOutput=False)
    if apply_ln1_affine:
        ln1w_p = nc.declare_dram_parameter("ln1w", [1, D], F32, isOutput=False)
        ln1b_p = nc.declare_dram_parameter("ln1b", [1, D], F32, isOutput=False)
    if apply_ln2_affine:
        ln2w_p = nc.declare_dram_parameter("ln2w", [1, D], F32, isOutput=False)
        ln2b_p = nc.declare_dram_parameter("ln2b", [1, D], F32, isOutput=False)
    out_p = nc.declare_dram_parameter("out", [RROWS, D], F32, isOutput=True)

    with tile.TileContext(nc) as tc:
        with contextlib.ExitStack() as stack:
            # ---------------- persistent pools ----------------
            const = stack.enter_context(tc.tile_pool(name="const", bufs=1))
            dram = stack.enter_context(tc.tile_pool(name="dram", bufs=1, space="DRAM"))
            y_pool = stack.enter_context(tc.tile_pool(name="y", bufs=1))
            y2T_pool = stack.enter_context(tc.tile_pool(name="y2T", bufs=1))

            eps_sb = const.tile([NP, 1], F32)
            nc.vector.memset(eps_sb[:], EPS)
            ones_f32 = const.tile([NP, 64], F32)
            nc.vector.memset(ones_f32[:], 1.0)
            ones_sb = const.tile([NP, 64], F32R)
            nc.vector.tensor_copy(out=ones_sb[:], in_=ones_f32[:])
            ident_sb = const.tile([NP, NP], BF16)
            nc.sync.dma_start(out=ident_sb[:], in_=ident_p[:, :])
            mask_sb = const.tile([NP, NP], BF16)
            nc.sync.dma_start(out=mask_sb[:], in_=mask_p[:, :])
            if add_bproj:
                bproj_bc = const.tile([NP, D], F32)
                nc.gpsimd.dma_start(out=bproj_bc[:], in_=bproj_p[:, :].to_broadcast((NP, D)))
            if add_bff2:
                bff2_bc = const.tile([NP, D], F32)
                nc.gpsimd.dma_start(out=bff2_bc[:], in_=bff2_p[:, :].to_broadcast((NP, D)))
            if add_bff1:
                bff1_sb = const.tile([NP, FC], F32)
                nc.sync.dma_start(out=bff1_sb[:], in_=bff1_p[:, :])
            if apply_ln1_affine:
                ln1w_bc = const.tile([NP, D], F32)
                nc.gpsimd.dma_start(out=ln1w_bc[:], in_=ln1w_p[:, :].to_broadcast((NP, D)))
                ln1b_bc = const.tile([NP, D], F32)
                nc.gpsimd.dma_start(out=ln1b_bc[:], in_=ln1b_p[:, :].to_broadcast((NP, D)))
            if apply_ln2_affine:
                ln2w_bc = const.tile([NP, D], F32)
                nc.gpsimd.dma_start(out=ln2w_bc[:], in_=ln2w_p[:, :].to_broadcast((NP, D)))
                ln2b_bc = const.tile([NP, D], F32)
                nc.gpsimd.dma_start(out=ln2b_bc[:], in_=ln2b_p[:, :].to_broadcast((NP, D)))

            # bf16 collective buffers, 2 chunks of 1024 rows each
            cc_in0 = dram.tile([RROWS, D], BF16)
            cc_in1 = dram.tile([RROWS, D], BF16)
            cc_out0 = dram.tile([RROWS // 2, D], BF16)
            cc_out1 = dram.tile([RROWS // 2, D], BF16)

            # y[0:4] / y2T persist into the FFN phase; half 0's residual+LN2
            # runs inside the fused attention phase (after RS#1 lands)
            y = [y_pool.tile([NP, D], F32, tag=f"y{rc}", name=f"y{rc}") for rc in range(4)]
            y2T = y2T_pool.tile([NP, DC, RROWS], BF16)

            # ---------------- fused prep + attention phase ----------------
            with contextlib.ExitStack() as att:
                qkT_pool = att.enter_context(tc.tile_pool(name="qkT", bufs=1))
                ctxT_pool = att.enter_context(tc.tile_pool(name="ctxT", bufs=1))
                wp_pool = att.enter_context(tc.tile_pool(name="wp", bufs=1))
                small = att.enter_context(tc.tile_pool(name="small", bufs=2))
                vaug_pool = att.enter_context(tc.tile_pool(name="vaug", bufs=1))
                e_pool = att.enter_context(tc.tile_pool(name="e", bufs=6))
                attn_pool = att.enter_context(tc.tile_pool(name="attn", bufs=2))
                wqkv = att.enter_context(tc.tile_pool(name="wqkv", bufs=1))
                hT_pool = att.enter_context(tc.tile_pool(name="hT", bufs=1))
                xln = att.enter_context(tc.tile_pool(name="xln", bufs=3))
                hrow = att.enter_context(tc.tile_pool(name="hrow", bufs=3))
                lnst = att.enter_context(tc.tile_pool(name="lnst", bufs=4))
                ywa = att.enter_context(tc.tile_pool(name="ywa", bufs=2))
                # PSUM: "sc" 2x[NP,2,512] (4 banks) + ctxA/ctxB (2) + aux (2)
                ctx_ps_pool = att.enter_context(tc.tile_pool(name="ctx_ps", bufs=1, space="PSUM"))
                sc_ps = att.enter_context(tc.tile_pool(name="sc_ps", bufs=2, space="PSUM"))
                aux_pool = att.enter_context(tc.tile_pool(name="aux", bufs=1, space="PSUM"))

                vaug = [vaug_pool.tile([NP, GH, HD + 1], BF16, tag=f"v{sc}", name=f"vaug{sc}") for sc in range(SC)]
                for sc in range(SC):
                    nc.vector.memset(vaug[sc][:, :, HD:HD + 1], 1.0)
                qT = [qkT_pool.tile([NP, S], BF16, tag=f"qT{p}", name=f"qT{p}") for p in range(4)]
                kT = [qkT_pool.tile([NP, S], BF16, tag=f"kT{p}", name=f"kT{p}") for p in range(4)]
                ctxT = [ctxT_pool.tile([NP, S], BF16, tag=f"ctxT{p}", name=f"ctxT{p}") for p in range(4)]
                wp_sb = wp_pool.tile([NP, 4, D], BF16)
                # LN1(x)^T, one 512-column group at a time (2 in flight)
                ht_tiles = {}

                def ht_tile(g):
                    if g not in ht_tiles:
                        ht_tiles[g] = hT_pool.tile([NP, DC, 512], BF16, tag="hT", bufs=2, name="hT_g")
                    return ht_tiles[g]
                # 2 PSUM banks multiplexed (half at a time, in emission order)
                # between q/k/v accumulators, h^T / y2^T PE transposes, proj
                # accumulators and softmax-denominator broadcasts.  Tile's
                # subtile dep tracking pipelines the two halves.
                aux = aux_pool.tile([NP, 2, 512], F32)
                aux_bf = [aux[:, h, :].bitcast(BF16) for h in range(2)]

                # group 0's x rows stream in BEFORE the (bulkier) qkv weights
                # so the LN pipeline starts immediately
                x_sbs = {}
                for sc in range(4):
                    x_sb = xln.tile([NP, D], F32, tag="x", name="x_sb")
                    for st in range(2):
                        nc.sync.dma_start(
                            out=x_sb[:, st * 512:(st + 1) * 512],
                            in_=x_p[sc * NP:(sc + 1) * NP, st * 512:(st + 1) * 512])
                    x_sbs[sc] = x_sb
                wv_sb = wqkv.tile([NP, DC, 512], BF16)
                wq_sb = wqkv.tile([NP, DC, 512], BF16)
                wk_sb = wqkv.tile([NP, DC, 512], BF16)
                nc.sync.dma_start(out=wv_sb[:], in_=wv_p[:, :, :])
                nc.sync.dma_start(out=wq_sb[:], in_=wq_p[:, :, :])
                nc.sync.dma_start(out=wk_sb[:], in_=wk_p[:, :, :])

                half_ctr = [0]

                def next_half():
                    h = half_ctr[0] & 1
                    half_ctr[0] += 1
                    return h

                def emit_ln_chunk(sc):
                    x_sb = x_sbs.pop(sc, None)
                    if x_sb is None:
                        x_sb = xln.tile([NP, D], F32, tag="x", name="x_sb")
                        for st in range(2):
                            nc.sync.dma_start(
                                out=x_sb[:, st * 512:(st + 1) * 512],
                                in_=x_p[sc * NP:(sc + 1) * NP, st * 512:(st + 1) * 512])
                    mv = _ln_stats(nc, lnst, x_sb[:], eps_sb)
                    h_sb = hrow.tile([NP, D], BF16, tag="h", name="h_sb")
                    nc.vector.tensor_scalar(
                        out=h_sb[:], in0=x_sb[:],
                        scalar1=mv[:, 0:1], scalar2=mv[:, 1:2],
                        op0=OP.subtract, op1=OP.mult)
                    if apply_ln1_affine:
                        nc.vector.tensor_mul(out=h_sb[:], in0=h_sb[:], in1=ln1w_bc[:])
                        nc.vector.tensor_add(out=h_sb[:], in0=h_sb[:], in1=ln1b_bc[:])
                    # all 8 d-chunk transposes land in ONE aux bank (bf16),
                    # evacuated by a single strided DVE copy
                    hb = next_half()
                    for dc in range(DC):
                        nc.tensor.transpose(
                            aux_bf[hb][:, dc * NP:(dc + 1) * NP],
                            h_sb[:, dc * NP:(dc + 1) * NP], ident_sb[:])
                    c4 = sc % 4
                    nc.vector.tensor_copy(
                        out=ht_tile(sc // 4)[:, :, c4 * NP:(c4 + 1) * NP],
                        in_=aux_bf[hb][:].rearrange("p (d c) -> p d c", d=DC))

                def emit_v(sc):
                    hb = next_half()
                    av = aux[:, hb, :]
                    hg = ht_tile(sc // 4)
                    c4 = sc % 4
                    for dc in range(DC):
                        nc.tensor.matmul(
                            av, hg[:, dc, c4 * NP:(c4 + 1) * NP], wv_sb[:, dc, :],
                            start=(dc == 0), stop=(dc == DC - 1))
                    nc.vector.tensor_copy(
                        out=vaug[sc][:, :, 0:HD],
                        in_=av.rearrange("p (h d) -> p h d", h=GH))

                def emit_qk(hp, w_sb, dst, s4):
                    hb = next_half()
                    av = aux[:, hb, :]
                    hg = ht_tile(s4)
                    for dc in range(DC):
                        nc.tensor.matmul(
                            av, w_sb[:, dc, hp * NP:(hp + 1) * NP], hg[:, dc, :],
                            start=(dc == 0), stop=(dc == DC - 1))
                    nc.vector.tensor_copy(out=dst[:, s4 * 512:(s4 + 1) * 512], in_=av)

                def emit_residual(rc):
                    # half-0 residual + LN2 + y2T transposes (rows rc*128..)
                    # DMAs ride the gpsimd queue: they gate on RS#1, and the
                    # sync queue must stay free for proj/cc_in1 traffic.
                    rs_sb = ywa.tile([NP, D], BF16, tag="rs", name="rs_sb")
                    nc.gpsimd.dma_start(out=rs_sb[:], in_=cc_out0[rc * NP:(rc + 1) * NP, :])
                    xr_sb = ywa.tile([NP, D], F32, tag="xr", name="xr_sb")
                    nc.gpsimd.dma_start(out=xr_sb[:], in_=xres_p[rc * NP:(rc + 1) * NP, :])
                    nc.vector.tensor_copy(out=y[rc][:], in_=rs_sb[:])
                    nc.vector.tensor_add(out=y[rc][:], in0=y[rc][:], in1=xr_sb[:])
                    if add_bproj:
                        nc.vector.tensor_add(out=y[rc][:], in0=y[rc][:], in1=bproj_bc[:])
                    mv = _ln_stats(nc, lnst, y[rc][:], eps_sb)
                    y2_sb = ywa.tile([NP, D], BF16, tag="y2", name="y2_sb")
                    nc.vector.tensor_scalar(
                        out=y2_sb[:], in0=y[rc][:],
                        scalar1=mv[:, 0:1], scalar2=mv[:, 1:2],
                        op0=OP.subtract, op1=OP.mult)
                    if apply_ln2_affine:
                        nc.vector.tensor_mul(out=y2_sb[:], in0=y2_sb[:], in1=ln2w_bc[:])
                        nc.vector.tensor_add(out=y2_sb[:], in0=y2_sb[:], in1=ln2b_bc[:])
                    hb = next_half()
                    for dc in range(DC):
                        nc.tensor.transpose(
                            aux_bf[hb][:, dc * NP:(dc + 1) * NP],
                            y2_sb[:, dc * NP:(dc + 1) * NP], ident_sb[:])
                    nc.vector.tensor_copy(
                        out=y2T[:, :, rc * NP:(rc + 1) * NP],
                        in_=aux_bf[hb][:].rearrange("p (d c) -> p d c", d=DC))

                # ---- group 0 runs eagerly (gates qc==0) ----
                for sc in range(4):
                    emit_ln_chunk(sc)
                for sc in range(4):
                    emit_v(sc)
                for hp in range(4):
                    emit_qk(hp, wq_sb, qT[hp], 0)
                    emit_qk(hp, wk_sb, kT[hp], 0)
                # wproj load deferred out of the startup DMA window (first
                # needed by emit_proj at qc==2)
                nc.sync.dma_start(out=wp_sb[:], in_=wproj_p[:, :, :])

                def emit_one_proj(qs, cc_dst, r):
                    attn_sb = attn_pool.tile([NP, D], BF16, tag="attnsb", name="attn_sb")
                    for nch in range(2):
                        hb = next_half()
                        av = aux[:, hb, :]
                        for pair in range(4):
                            nc.tensor.matmul(
                                av,
                                ctxT[pair][:, qs * NP:(qs + 1) * NP],
                                wp_sb[:, pair, nch * 512:(nch + 1) * 512],
                                start=(pair == 0), stop=(pair == 3))
                        nc.vector.tensor_copy(out=attn_sb[:, nch * 512:(nch + 1) * 512], in_=av)
                    nc.sync.dma_start(out=cc_dst[r * NP:(r + 1) * NP, :], in_=attn_sb[:])

                pending = deque()
                pending_late = deque()
                pending_norm = []

                def flush_norm():
                    while pending_norm:
                        pending_norm.pop(0)()

                def make_norm(ctx_ps, hp, po, qbase):
                    def emit():
                        den = small.tile([NP, 512], F32R, tag="den", name="den")
                        nc.vector.tensor_copy(out=den[64:65, :], in_=ctx_ps[64:65, :])
                        hb = next_half()
                        b_ps = aux[0:64, hb, :]
                        nc.tensor.matmul(b_ps, ones_sb[64:65, :], den[64:65, :],
                                         start=True, stop=True)
                        b_sb = small.tile([64, 512], F32, tag="bsb", name="b_sb")
                        nc.vector.reciprocal_approx_fast(out=b_sb[:], in_=b_ps)
                        nc.vector.tensor_mul(
                            out=ctxT[hp][po:po + 64, qbase:qbase + 512],
                            in0=ctx_ps[0:64, :], in1=b_sb[:])
                    return emit

                for qc in range(QC):
                    qbase = qc * 512
                    kcs = [4 * qc] + list(range(0, 4 * qc)) + [4 * qc + 1, 4 * qc + 2, 4 * qc + 3]
                    nsteps = 4 * len(kcs)
                    qstep = 0
                    # group qc+1's prep drips into this qc's attention steps
                    if qc < 3:
                        g = qc + 1
                        for sc in range(4 * g, 4 * g + 4):
                            pending.append(lambda sc=sc: emit_ln_chunk(sc))
                        for sc in range(4 * g, 4 * g + 4):
                            pending.append(lambda sc=sc: emit_v(sc))
                        for hp in range(4):
                            pending.append(lambda hp=hp, g=g: emit_qk(hp, wq_sb, qT[hp], g))
                            pending.append(lambda hp=hp, g=g: emit_qk(hp, wk_sb, kT[hp], g))
                    for hp in range(4):
                        ctxA = ctx_ps_pool.tile([HD + 1, 512], F32, tag="ctxA", name="ctxA")
                        ctxB = ctx_ps_pool.tile([HD + 1, 512], F32, tag="ctxB", name="ctxB")
                        pend = None
                        for i, kc in enumerate(kcs):
                            off = max(0, 128 * kc - qbase)
                            s2 = sc_ps.tile([NP, 2, 512], F32, tag="sc", name="s2")
                            # concurrent in the PE array: row groups 0-63 / 64-127
                            nc.tensor.matmul(
                                s2[:, 0, off:512],
                                kT[hp][0:64, kc * NP:(kc + 1) * NP],
                                qT[hp][0:64, qbase + off:qbase + 512],
                                start=True, stop=True)
                            nc.tensor.matmul(
                                s2[:, 1, off:512],
                                kT[hp][64:128, kc * NP:(kc + 1) * NP],
                                qT[hp][64:128, qbase + off:qbase + 512],
                                start=True, stop=True)
                            e2 = e_pool.tile([NP, 2, 512], BF16, tag="esb", name="e2")
                            # ONE activation over both heads' PSUM banks:
                            # halves the per-step ACTIVATE dispatch overhead
                            nc.scalar.activation(out=e2[:, :, off:512], in_=s2[:, :, off:512], func=AF.Exp)
                            if 4 * qc <= kc:
                                nc.vector.tensor_mul(
                                    out=e2[:, 0, off:off + 128], in0=e2[:, 0, off:off + 128], in1=mask_sb[:])
                                nc.vector.tensor_mul(
                                    out=e2[:, 1, off:off + 128], in0=e2[:, 1, off:off + 128], in1=mask_sb[:])
                            qstep += 1
                            if i == 0:
                                # previous pair's softmax normalization, deferred
                                # past this pair's first scores/exp so the
                                # scalar-engine exp stream never stalls at the
                                # pair boundary (must drain before this pair's
                                # first ctx matmul reuses the ctx banks)
                                flush_norm()
                            elif pending:
                                pending.popleft()()
                            elif pending_late and qstep * 4 >= nsteps * 3:
                                # RS#1-gated work: only in the last quarter of
                                # the qc, by which time the collective is done
                                # (a PE transpose gated on an unfinished
                                # collective would stall the whole PE queue)
                                pending_late.popleft()()
                            if pend is not None:
                                off_, e2_, kc_, first_ = pend
                                nc.tensor.matmul(
                                    ctxA[:, off_:512], vaug[kc_][:, 2 * hp, :],
                                    e2_[:, 0, off_:512], start=first_, stop=False)
                                nc.tensor.matmul(
                                    ctxB[:, off_:512], vaug[kc_][:, 2 * hp + 1, :],
                                    e2_[:, 1, off_:512], start=first_, stop=False)
                            pend = (off, e2, kc, i == 0)
                        off_, e2_, kc_, first_ = pend
                        nc.tensor.matmul(
                            ctxA[:, off_:512], vaug[kc_][:, 2 * hp, :],
                            e2_[:, 0, off_:512], start=first_, stop=True)
                        nc.tensor.matmul(
                            ctxB[:, off_:512], vaug[kc_][:, 2 * hp + 1, :],
                            e2_[:, 1, off_:512], start=first_, stop=True)
                        pending_norm.append(make_norm(ctxA, hp, 0, qbase))
                        pending_norm.append(make_norm(ctxB, hp, 64, qbase))

                    while pending:
                        pending.popleft()()
                    if qc == 1:
                        pending.extend(
                            (lambda qs=qs: emit_one_proj(qs, cc_in0, qs))
                            for qs in range(8))
                    if qc == 2:
                        nc.gpsimd.collective_compute(
                            "ReduceScatter", OP.add,
                            replica_groups=[[0, 1], [2, 3], [4, 5], [6, 7]],
                            ins=[cc_in0[:].opt()], outs=[cc_out0[:].opt()])
                        # qs 8-11 (rows 1024-1535) only need qc<=2 context;
                        # interleave them into qc==3's attention steps
                        pending.extend(
                            (lambda qs=qs: emit_one_proj(qs, cc_in1, qs - 8))
                            for qs in range(8, 12))
                        # FFN half-0 prologue, gated on RS#1: drip LATE in qc3
                        pending_late.extend(
                            (lambda rc=rc: emit_residual(rc)) for rc in range(4))
                    if qc == 3:
                        flush_norm()
                        for qs in range(12, 16):
                            emit_one_proj(qs, cc_in1, qs - 8)
                        while pending_late:
                            pending_late.popleft()()

            # RS#2 issued OUTSIDE the attention pool scopes: the pool-stack
            # close drains all engines, so a collective issued inside would
            # serialize the whole FFN behind its completion.
            nc.gpsimd.collective_compute(
                "ReduceScatter", OP.add,
                replica_groups=[[0, 1], [2, 3], [4, 5], [6, 7]],
                ins=[cc_in1[:].opt()], outs=[cc_out1[:].opt()])

            # ---------------- FFN phase (1024 rows per core) ----------------
            with contextlib.ExitStack() as ffn:
                g_pool = ffn.enter_context(tc.tile_pool(name="g", bufs=64))
                y47_pool = ffn.enter_context(tc.tile_pool(name="y47", bufs=1))
                yw = ffn.enter_context(tc.tile_pool(name="yw", bufs=3))
                lnst2 = ffn.enter_context(tc.tile_pool(name="lnst2", bufs=4))
                w1_pool = ffn.enter_context(tc.tile_pool(name="w1", bufs=10))
                w2_pool = ffn.enter_context(tc.tile_pool(name="w2", bufs=8))
                out_pool = ffn.enter_context(tc.tile_pool(name="outp", bufs=3))
                ff_ps_pool = ffn.enter_context(tc.tile_pool(name="ff_ps", bufs=2, space="PSUM"))
                z_ps_pool = ffn.enter_context(tc.tile_pool(name="z_ps", bufs=1, space="PSUM"))
                tp_ps = ffn.enter_context(tc.tile_pool(name="tp_ffn", bufs=2, space="PSUM"))

                ys = y + [y47_pool.tile([NP, D], F32, tag=f"y{rc}", name=f"y{rc}") for rc in range(4, 8)]
                for half in range(2):
                    if half == 1:
                        # residual + LN2 for half 1 (half 0 ran inside the
                        # fused phase); DMAs on gpsimd so the RS#2 gate never
                        # blocks w1/w2 streaming on the sync queue
                        for r4 in range(4):
                            rc = 4 + r4
                            rs_sb = yw.tile([NP, D], BF16, tag="rs")
                            nc.gpsimd.dma_start(out=rs_sb[:], in_=cc_out1[r4 * NP:(r4 + 1) * NP, :])
                            xr_sb = yw.tile([NP, D], F32, tag="xr")
                            nc.gpsimd.dma_start(out=xr_sb[:], in_=xres_p[rc * NP:(rc + 1) * NP, :])
                            nc.vector.tensor_copy(out=ys[rc][:], in_=rs_sb[:])
                            nc.vector.tensor_add(out=ys[rc][:], in0=ys[rc][:], in1=xr_sb[:])
                            if add_bproj:
                                nc.vector.tensor_add(out=ys[rc][:], in0=ys[rc][:], in1=bproj_bc[:])
                            mv = _ln_stats(nc, lnst2, ys[rc][:], eps_sb)
                            y2_sb = yw.tile([NP, D], BF16, tag="y2")
                            nc.vector.tensor_scalar(
                                out=y2_sb[:], in0=ys[rc][:],
                                scalar1=mv[:, 0:1], scalar2=mv[:, 1:2],
                                op0=OP.subtract, op1=OP.mult)
                            if apply_ln2_affine:
                                nc.vector.tensor_mul(out=y2_sb[:], in0=y2_sb[:], in1=ln2w_bc[:])
                                nc.vector.tensor_add(out=y2_sb[:], in0=y2_sb[:], in1=ln2b_bc[:])
                            for dc in range(DC):
                                t_ps = tp_ps.tile([NP, NP], BF16, tag="tp")
                                nc.tensor.transpose(t_ps[:], y2_sb[:, dc * NP:(dc + 1) * NP], ident_sb[:])
                                nc.scalar.copy(out=y2T[:, dc, rc * NP:(rc + 1) * NP], in_=t_ps[:])

                    hcols = slice(half * 512, (half + 1) * 512)
                    gts = []
                    for ffc in range(FC):
                        w1_sb = w1_pool.tile([NP, DC * NP], BF16, tag="w1")
                        nc.sync.dma_start(out=w1_sb[:], in_=wff1_p[ffc, :, :])
                        ff_ps = ff_ps_pool.tile([NP, 512], F32, tag="ffps")
                        for dc in range(DC):
                            nc.tensor.matmul(
                                ff_ps[:],
                                w1_sb[:, dc * NP:(dc + 1) * NP],
                                y2T[:, dc, hcols],
                                start=(dc == 0), stop=(dc == DC - 1))
                        g_sb = g_pool.tile([NP, 512], BF16, tag="g")
                        nc.scalar.activation(out=g_sb[:], in_=ff_ps[:], func=AF.Gelu,
                                             bias=(bff1_sb[:, ffc:ffc + 1] if add_bff1 else 0.0),
                                             scale=1.0)
                        gts.append(g_sb)
                    for nch in range(2):
                        ncols = slice(nch * 512, (nch + 1) * 512)
                        z_pss = [z_ps_pool.tile([NP, 512], F32, tag=f"zps{r}", name=f"zps{r}") for r in range(4)]
                        for ffc in range(FC):
                            w2_sb = w2_pool.tile([NP, 512], BF16, tag="w2")
                            nc.sync.dma_start(out=w2_sb[:], in_=wff2_p[ffc, :, ncols])
                            for r4 in range(4):
                                nc.tensor.matmul(
                                    z_pss[r4][:],
                                    gts[ffc][:, r4 * NP:(r4 + 1) * NP],
                                    w2_sb[:],
                                    start=(ffc == 0), stop=(ffc == FC - 1))
                        for r4 in range(4):
                            rc = half * 4 + r4
                            o_sb = out_pool.tile([NP, 512], F32, tag="osb")
                            nc.vector.tensor_add(out=o_sb[:], in0=z_pss[r4][:], in1=ys[rc][:, ncols])
                            if add_bff2:
                                nc.vector.tensor_add(out=o_sb[:], in0=o_sb[:], in1=bff2_bc[:, ncols])
                            nc.sync.dma_start(out=out_p[rc * NP:(rc + 1) * NP, ncols], in_=o_sb[:])

    nc.compile()
    return nc


# ------------------------- host-side driver -------------------------

_BF = ml_dtypes.bfloat16


def _core_rows(g):
    return np.r_[512 * g:512 * g + 512, 1024 + 512 * g:1536 + 512 * g]


def _prep_core_inputs(inputs, b, g, flags):
    x = np.asarray(inputs["x"], np.float32)
    w_qkv = np.asarray(inputs["w_qkv"], np.float32).reshape(D, H, HD, 3)
    hs = slice(g * GH, (g + 1) * GH)
    w_k = w_qkv[:, hs, :, 0].reshape(D, GH * HD)
    w_q = (w_qkv[:, hs, :, 1] * (HD ** -0.5)).reshape(D, GH * HD)
    w_v = w_qkv[:, hs, :, 2].reshape(D, GH * HD)

    def tile_kxm(w):  # [D, 512] -> [128, DC, 512]
        return np.ascontiguousarray(w.reshape(DC, NP, GH * HD).transpose(1, 0, 2))

    w_proj = np.asarray(inputs["w_proj"], np.float32)
    wp = np.ascontiguousarray(
        w_proj[g * 512:(g + 1) * 512, :].reshape(4, NP, D).transpose(1, 0, 2))

    w_ff1 = np.asarray(inputs["w_ff1"], np.float32)
    w1t = np.ascontiguousarray(
        w_ff1.reshape(DC, NP, FC, NP).transpose(2, 1, 0, 3).reshape(FC, NP, DC * NP))
    w_ff2 = np.asarray(inputs["w_ff2"], np.float32)
    w2t = np.ascontiguousarray(w_ff2.reshape(FC, NP, D))

    j = np.arange(NP)[:, None]
    i = np.arange(NP)[None, :]
    mask = (j <= i).astype(np.float32)

    m = {
        "x": np.ascontiguousarray(x[b]),
        "x_res": np.ascontiguousarray(x[b][_core_rows(g)]),
        "wq": tile_kxm(w_q).astype(_BF),
        "wk": tile_kxm(w_k).astype(_BF),
        "wv": tile_kxm(w_v).astype(_BF),
        "wproj": wp.astype(_BF),
        "wff1": w1t.astype(_BF),
        "wff2": w2t.astype(_BF),
        "ident": np.eye(NP, dtype=_BF),
        "mask": mask.astype(_BF),
    }
    if flags[0]:
        m["bproj"] = np.asarray(inputs["b_proj"], np.float32).reshape(1, D).copy()
    if flags[1]:
        m["bff1"] = np.ascontiguousarray(np.asarray(inputs["b_ff1"], np.float32).reshape(FC, NP).T)
    if flags[2]:
        m["bff2"] = np.asarray(inputs["b_ff2"], np.float32).reshape(1, D).copy()
    return m


_NC_CACHE = {}


def kernel(**inputs):
    ln1w = np.asarray(inputs["ln1_w"], np.float32)
    ln1b = np.asarray(inputs["ln1_b"], np.float32)
    ln2w = np.asarray(inputs["ln2_w"], np.float32)
    ln2b = np.asarray(inputs["ln2_b"], np.float32)
    a1 = not (np.allclose(ln1w, 1.0) and np.allclose(ln1b, 0.0))
    a2 = not (np.allclose(ln2w, 1.0) and np.allclose(ln2b, 0.0))
    bp = not np.allclose(np.asarray(inputs["b_proj"], np.float32), 0.0)
    b1 = not np.allclose(np.asarray(inputs["b_ff1"], np.float32), 0.0)
    b2 = not np.allclose(np.asarray(inputs["b_ff2"], np.float32), 0.0)
    flags = (bp, b1, b2)

    key = (a1, a2, bp, b1, b2)
    if key not in _NC_CACHE:
        _NC_CACHE[key] = build(apply_ln1_affine=a1, apply_ln2_affine=a2,
                               add_bproj=bp, add_bff1=b1, add_bff2=b2)
    nc = _NC_CACHE[key]

    in_maps = []
    for core in range(8):
        b, g = core // 2, core % 2
        m = _prep_core_inputs(inputs, b, g, flags)
        if a1:
            m["ln1w"] = ln1w.reshape(1, D).copy()
            m["ln1b"] = ln1b.reshape(1, D).copy()
        if a2:
            m["ln2w"] = ln2w.reshape(1, D).copy()
            m["ln2b"] = ln2b.reshape(1, D).copy()
        in_maps.append(m)

    res = run_bass_kernel_spmd(nc, in_maps, core_ids=list(range(8)))

    out = np.empty((B, S, D), np.float32)
    for core in range(8):
        b, g = core // 2, core % 2
        out[b][_core_rows(g)] = res.results[core]["out"]
    return out


# revision 17
# speedup vs baseline: 1.0167x; 1.0167x over previous
"""Trainium2 Bass kernel for a pre-LN transformer block (attention + FFN).

Sharding over 8 NeuronCores: core (b, g) = batch b (0..3) x head-group g (0..1).
Each core runs LN1 + QKV (its 8 heads) + causal attention + its slice of the
output projection for its batch; a pairwise bf16 ReduceScatter (2 chunks,
first overlapped with attention of the second query half) sums the two
head-groups' partial attn_out; each core then finishes 1024 rows
(residual + LN2 + full FFN).

The prep pipeline (LN1 + h^T transposes + q/k/v projections) is FUSED into
the attention qc loop: attention is scalar-engine-bound (the exp stream),
so group g+1's prep matmuls are dripped one-per-step into qc g's attention
steps where they soak up the idle PE time.  Likewise the FFN half-0
residual+LN2+transpose prologue is dripped into late qc==3 (after the first
ReduceScatter has landed), so FF1 can start the moment attention drains.

Attention inner loop: the two heads of a q/k pair live on partitions 0:64 and
64:128, so their K=64 score matmuls run CONCURRENTLY in the PE array via
row-group tiling; both heads' scores land in one 2-bank PSUM tile so a
SINGLE scalar-engine exp (the critical resource) covers both.  Softmax
denominators via a ones-column on V and reciprocal_approx_fast.
"""
import sys

if "/opt/trn_rl_repo" not in sys.path:
    sys.path.insert(0, "/opt/trn_rl_repo")

import contextlib
from collections import deque

import numpy as np
import ml_dtypes

import concourse.bass as bass
import concourse.bacc as bacc

# Force Exp and Ln activations onto the ONE table set that contains both
# (natural_log_exp_and_others): the set-assignment pass otherwise puts Ln
# and Exp in different sets and every LayerNorm interleaved into the
# attention exp stream costs two ~2.7us ACT_TABLE_LOADs.  Canonical set
# indices are preserved (sets are edited in place, not reordered).
_orig_gat = bacc.get_activation_tables


def _gat_force_ln_exp(arch):
    tables = _orig_gat(arch)
    exp = mybir.ActivationFunctionType.Exp
    ln = mybir.ActivationFunctionType.Ln
    for name, fns in tables.items():
        if name != "natural_log_exp_and_others":
            fns.discard(exp)
            fns.discard(ln)
    return tables


bacc.get_activation_tables = _gat_force_ln_exp
import concourse.tile as tile
from concourse import mybir
from concourse.bass_utils import run_bass_kernel_spmd

F32 = mybir.dt.float32
F32R = mybir.dt.float32r
BF16 = mybir.dt.bfloat16
AF = mybir.ActivationFunctionType
OP = mybir.AluOpType

B, S, D, H = 4, 2048, 1024, 16
HD = D // H
FF = 4 * D
EPS = 1e-5
GH = 8          # heads per core
NP = 128        # partitions
SC = S // NP    # 16 seq chunks of 128
DC = D // NP    # 8 d-chunks
QC = S // 512   # 4 q-chunks of 512
FC = FF // NP   # 32 ff chunks of 128
RROWS = S // 2  # 1024 rows finalized per core


def _ln_stats(nc, pool, x_ap, eps_sb):
    """bn_stats/bn_aggr mean+rstd for a [128, D] fp32 tile. Returns mv tile;
    mv[:,0:1]=mean, mv[:,1:2]=rstd.

    rstd = exp(-0.5*ln(var+eps)): Ln and Exp live in the SAME activation
    table set (natural_log_exp_and_others) as the attention/softmax Exp, so
    LN chunks interleaved into the exp stream cost no ACT_TABLE_LOAD (~2.7us
    per switch) — Sqrt would force a table swap both ways every chunk."""
    stats = pool.tile([NP, 2, 6], F32, tag="lnstats")
    nc.vector.bn_stats(out=stats[:, 0, :], in_=x_ap[:, 0:512])
    nc.vector.bn_stats(out=stats[:, 1, :], in_=x_ap[:, 512:1024])
    mv = pool.tile([NP, 2], F32, tag="lnmv")
    nc.vector.bn_aggr(out=mv[:], in_=stats[:])
    nc.scalar.activation(out=mv[:, 1:2], in_=mv[:, 1:2], func=AF.Ln,
                         bias=eps_sb[:], scale=1.0)
    nc.scalar.activation(out=mv[:, 1:2], in_=mv[:, 1:2], func=AF.Exp,
                         bias=0.0, scale=-0.5)
    return mv


def build(apply_ln1_affine=False, apply_ln2_affine=False,
          add_bproj=False, add_bff1=False, add_bff2=False):
    nc = bacc.Bacc("TRN2", num_devices=8)

    # ---- DRAM parameters (per-core shards, laid out host-side) ----
    x_p = nc.declare_dram_parameter("x", [S, D], F32, isOutput=False)
    xres_p = nc.declare_dram_parameter("x_res", [RROWS, D], F32, isOutput=False)
    wq_p = nc.declare_dram_parameter("wq", [NP, DC, 512], BF16, isOutput=False)
    wk_p = nc.declare_dram_parameter("wk", [NP, DC, 512], BF16, isOutput=False)
    wv_p = nc.declare_dram_parameter("wv", [NP, DC, 512], BF16, isOutput=False)
    wproj_p = nc.declare_dram_parameter("wproj", [NP, 4, D], BF16, isOutput=False)
    wff1_p = nc.declare_dram_parameter("wff1", [FC, NP, DC * NP], BF16, isOutput=False)
    wff2_p = nc.declare_dram_parameter("wff2", [FC, NP, D], BF16, isOutput=False)
    if add_bproj:
        bproj_p = nc.declare_dram_parameter("bproj", [1, D], F32, isOutput=False)
    if add_bff1:
        bff1_p = nc.declare_dram_parameter("bff1", [NP, FC], F32, isOutput=False)
    if add_bff2:
        bff2_p = nc.declare_dram_parameter("bff2", [1, D], F32, isOutput=False)
    ident_p = nc.declare_dram_parameter("ident", [NP, NP], BF16, isOutput=False)
    mask_p = nc.declare_dram_parameter("mask", [NP, NP], BF16, isOutput=False)
    if apply_ln1_affine:
        ln1w_p = nc.declare_dram_parameter("ln1w", [1, D], F32, isOutput=False)
        ln1b_p = nc.declare_dram_parameter("ln1b", [1, D], F32, isOutput=False)
    if apply_ln2_affine:
        ln2w_p = nc.declare_dram_parameter("ln2w", [1, D], F32, isOutput=False)
        ln2b_p = nc.declare_dram_parameter("ln2b", [1, D], F32, isOutput=False)
    out_p = nc.declare_dram_parameter("out", [RROWS, D], F32, isOutput=True)

    with tile.TileContext(nc) as tc:
        with contextlib.ExitStack() as stack:
            # ---------------- persistent pools ----------------
            const = stack.enter_context(tc.tile_pool(name="const", bufs=1))
            dram = stack.enter_context(tc.tile_pool(name="dram", bufs=1, space="DRAM"))
            y_pool = stack.enter_context(tc.tile_pool(name="y", bufs=1))
            y2T_pool = stack.enter_context(tc.tile_pool(name="y2T", bufs=1))

            eps_sb = const.tile([NP, 1], F32)
            nc.vector.memset(eps_sb[:], EPS)
            ones_f32 = const.tile([NP, 64], F32)
            nc.vector.memset(ones_f32[:], 1.0)
            ones_sb = const.tile([NP, 64], F32R)
            nc.vector.tensor_copy(out=ones_sb[:], in_=ones_f32[:])
            ident_sb = const.tile([NP, NP], BF16)
            nc.sync.dma_start(out=ident_sb[:], in_=ident_p[:, :])
            mask_sb = const.tile([NP, NP], BF16)
            nc.sync.dma_start(out=mask_sb[:], in_=mask_p[:, :])
            if add_bproj:
                bproj_bc = const.tile([NP, D], F32)
                nc.gpsimd.dma_start(out=bproj_bc[:], in_=bproj_p[:, :].to_broadcast((NP, D)))
            if add_bff2:
                bff2_bc = const.tile([NP, D], F32)
                nc.gpsimd.dma_start(out=bff2_bc[:], in_=bff2_p[:, :].to_broadcast((NP, D)))
            if add_bff1:
                bff1_sb = const.tile([NP, FC], F32)
                nc.sync.dma_start(out=bff1_sb[:], in_=bff1_p[:, :])
            if apply_ln1_affine:
                ln1w_bc = const.tile([NP, D], F32)
                nc.gpsimd.dma_start(out=ln1w_bc[:], in_=ln1w_p[:, :].to_broadcast((NP, D)))
                ln1b_bc = const.tile([NP, D], F32)
                nc.gpsimd.dma_start(out=ln1b_bc[:], in_=ln1b_p[:, :].to_broadcast((NP, D)))
            if apply_ln2_affine:
                ln2w_bc = const.tile([NP, D], F32)
                nc.gpsimd.dma_start(out=ln2w_bc[:], in_=ln2w_p[:, :].to_broadcast((NP, D)))
                ln2b_bc = const.tile([NP, D], F32)
                nc.gpsimd.dma_start(out=ln2b_bc[:], in_=ln2b_p[:, :].to_broadcast((NP, D)))

            # bf16 collective buffers, 2 chunks of 1024 rows each
            cc_in0 = dram.tile([RROWS, D], BF16)
            cc_in1 = dram.tile([RROWS, D], BF16)
            cc_out0 = dram.tile([RROWS // 2, D], BF16)
            cc_out1 = dram.tile([RROWS // 2, D], BF16)

            # y[0:4] / y2T persist into the FFN phase; half 0's residual+LN2
            # runs inside the fused attention phase (after RS#1 lands)
            y = [y_pool.tile([NP, D], F32, tag=f"y{rc}", name=f"y{rc}") for rc in range(4)]
            y2T = y2T_pool.tile([NP, DC, RROWS], BF16)

            # ---------------- fused prep + attention phase ----------------
            with contextlib.ExitStack() as att:
                qkT_pool = att.enter_context(tc.tile_pool(name="qkT", bufs=1))
                ctxT_pool = att.enter_context(tc.tile_pool(name="ctxT", bufs=1))
                wp_pool = att.enter_context(tc.tile_pool(name="wp", bufs=1))
                small = att.enter_context(tc.tile_pool(name="small", bufs=2))
                vaug_pool = att.enter_context(tc.tile_pool(name="vaug", bufs=1))
                e_pool = att.enter_context(tc.tile_pool(name="e", bufs=6))
                attn_pool = att.enter_context(tc.tile_pool(name="attn", bufs=2))
                wqkv = att.enter_context(tc.tile_pool(name="wqkv", bufs=1))
                hT_pool = att.enter_context(tc.tile_pool(name="hT", bufs=1))
                xln = att.enter_context(tc.tile_pool(name="xln", bufs=3))
                hrow = att.enter_context(tc.tile_pool(name="hrow", bufs=3))
                lnst = att.enter_context(tc.tile_pool(name="lnst", bufs=4))
                ywa = att.enter_context(tc.tile_pool(name="ywa", bufs=2))
                # PSUM: "sc" 2x[NP,2,512] (4 banks) + ctxA/ctxB (2) + aux (2)
                ctx_ps_pool = att.enter_context(tc.tile_pool(name="ctx_ps", bufs=1, space="PSUM"))
                sc_ps = att.enter_context(tc.tile_pool(name="sc_ps", bufs=2, space="PSUM"))
                aux_pool = att.enter_context(tc.tile_pool(name="aux", bufs=1, space="PSUM"))

                vaug = [vaug_pool.tile([NP, GH, HD + 1], BF16, tag=f"v{sc}", name=f"vaug{sc}") for sc in range(SC)]
                for sc in range(SC):
                    nc.vector.memset(vaug[sc][:, :, HD:HD + 1], 1.0)
                qT = [qkT_pool.tile([NP, S], BF16, tag=f"qT{p}", name=f"qT{p}") for p in range(4)]
                kT = [qkT_pool.tile([NP, S], BF16, tag=f"kT{p}", name=f"kT{p}") for p in range(4)]
                ctxT = [ctxT_pool.tile([NP, S], BF16, tag=f"ctxT{p}", name=f"ctxT{p}") for p in range(4)]
                wp_sb = wp_pool.tile([NP, 4, D], BF16)
                # LN1(x)^T, one 512-column group at a time (2 in flight)
                ht_tiles = {}

                def ht_tile(g):
                    if g not in ht_tiles:
                        ht_tiles[g] = hT_pool.tile([NP, DC, 512], BF16, tag="hT", bufs=2, name="hT_g")
                    return ht_tiles[g]
                # 2 PSUM banks multiplexed (half at a time, in emission order)
                # between q/k/v accumulators, h^T / y2^T PE transposes, proj
                # accumulators and softmax-denominator broadcasts.  Tile's
                # subtile dep tracking pipelines the two halves.
                aux = aux_pool.tile([NP, 2, 512], F32)
                aux_bf = [aux[:, h, :].bitcast(BF16) for h in range(2)]

                # group 0's x rows stream in BEFORE the (bulkier) qkv weights
                # so the LN pipeline starts immediately
                x_sbs = {}
                for sc in range(4):
                    x_sb = xln.tile([NP, D], F32, tag="x", name="x_sb")
                    for st in range(2):
                        nc.sync.dma_start(
                            out=x_sb[:, st * 512:(st + 1) * 512],
                            in_=x_p[sc * NP:(sc + 1) * NP, st * 512:(st + 1) * 512])
                    x_sbs[sc] = x_sb
                wv_sb = wqkv.tile([NP, DC, 512], BF16)
                wq_sb = wqkv.tile([NP, DC, 512], BF16)
                wk_sb = wqkv.tile([NP, DC, 512], BF16)
                nc.sync.dma_start(out=wv_sb[:], in_=wv_p[:, :, :])
                nc.sync.dma_start(out=wq_sb[:], in_=wq_p[:, :, :])
                nc.sync.dma_start(out=wk_sb[:], in_=wk_p[:, :, :])

                half_ctr = [0]

                def next_half():
                    h = half_ctr[0] & 1
                    half_ctr[0] += 1
                    return h

                def emit_x(sc):
                    x_sb = xln.tile([NP, D], F32, tag="x", name="x_sb")
                    for st in range(2):
                        nc.sync.dma_start(
                            out=x_sb[:, st * 512:(st + 1) * 512],
                            in_=x_p[sc * NP:(sc + 1) * NP, st * 512:(st + 1) * 512])
                    x_sbs[sc] = x_sb

                def emit_ln_chunk(sc):
                    x_sb = x_sbs.pop(sc, None)
                    if x_sb is None:
                        x_sb = xln.tile([NP, D], F32, tag="x", name="x_sb")
                        for st in range(2):
                            nc.sync.dma_start(
                                out=x_sb[:, st * 512:(st + 1) * 512],
                                in_=x_p[sc * NP:(sc + 1) * NP, st * 512:(st + 1) * 512])
                    mv = _ln_stats(nc, lnst, x_sb[:], eps_sb)
                    h_sb = hrow.tile([NP, D], BF16, tag="h", name="h_sb")
                    nc.vector.tensor_scalar(
                        out=h_sb[:], in0=x_sb[:],
                        scalar1=mv[:, 0:1], scalar2=mv[:, 1:2],
                        op0=OP.subtract, op1=OP.mult)
                    if apply_ln1_affine:
                        nc.vector.tensor_mul(out=h_sb[:], in0=h_sb[:], in1=ln1w_bc[:])
                        nc.vector.tensor_add(out=h_sb[:], in0=h_sb[:], in1=ln1b_bc[:])
                    # all 8 d-chunk transposes land in ONE aux bank (bf16),
                    # evacuated by a single strided DVE copy
                    hb = next_half()
                    for dc in range(DC):
                        nc.tensor.transpose(
                            aux_bf[hb][:, dc * NP:(dc + 1) * NP],
                            h_sb[:, dc * NP:(dc + 1) * NP], ident_sb[:])
                    c4 = sc % 4
                    nc.vector.tensor_copy(
                        out=ht_tile(sc // 4)[:, :, c4 * NP:(c4 + 1) * NP],
                        in_=aux_bf[hb][:].rearrange("p (d c) -> p d c", d=DC))

                def emit_v(sc):
                    hb = next_half()
                    av = aux[:, hb, :]
                    hg = ht_tile(sc // 4)
                    c4 = sc % 4
                    for dc in range(DC):
                        nc.tensor.matmul(
                            av, hg[:, dc, c4 * NP:(c4 + 1) * NP], wv_sb[:, dc, :],
                            start=(dc == 0), stop=(dc == DC - 1))
                    nc.scalar.copy(
                        out=vaug[sc][:, :, 0:HD],
                        in_=av.rearrange("p (h d) -> p h d", h=GH))

                def emit_qk(hp, w_sb, dst, s4):
                    hb = next_half()
                    av = aux[:, hb, :]
                    hg = ht_tile(s4)
                    for dc in range(DC):
                        nc.tensor.matmul(
                            av, w_sb[:, dc, hp * NP:(hp + 1) * NP], hg[:, dc, :],
                            start=(dc == 0), stop=(dc == DC - 1))
                    nc.scalar.copy(out=dst[:, s4 * 512:(s4 + 1) * 512], in_=av)

                res_y2 = {}

                def emit_residual_dve(rc):
                    # half-0 residual + LN2 (rows rc*128..): DMA + DVE only,
                    # no PE instruction — safe to drip once RS#1 has landed
                    rs_sb = ywa.tile([NP, D], BF16, tag="rs", name="rs_sb")
                    nc.sync.dma_start(out=rs_sb[:], in_=cc_out0[rc * NP:(rc + 1) * NP, :])
                    xr_sb = ywa.tile([NP, D], F32, tag="xr", name="xr_sb")
                    nc.sync.dma_start(out=xr_sb[:], in_=xres_p[rc * NP:(rc + 1) * NP, :])
                    nc.vector.tensor_copy(out=y[rc][:], in_=rs_sb[:])
                    nc.vector.tensor_add(out=y[rc][:], in0=y[rc][:], in1=xr_sb[:])
                    if add_bproj:
                        nc.vector.tensor_add(out=y[rc][:], in0=y[rc][:], in1=bproj_bc[:])
                    mv = _ln_stats(nc, lnst, y[rc][:], eps_sb)
                    y2_sb = ywa.tile([NP, D], BF16, tag="y2", name="y2_sb")
                    nc.vector.tensor_scalar(
                        out=y2_sb[:], in0=y[rc][:],
                        scalar1=mv[:, 0:1], scalar2=mv[:, 1:2],
                        op0=OP.subtract, op1=OP.mult)
                    if apply_ln2_affine:
                        nc.vector.tensor_mul(out=y2_sb[:], in0=y2_sb[:], in1=ln2w_bc[:])
                        nc.vector.tensor_add(out=y2_sb[:], in0=y2_sb[:], in1=ln2b_bc[:])
                    res_y2[rc] = y2_sb

                def emit_residual_pe(rc):
                    # the PE transposes, dripped a few steps after the DVE
                    # part so they never head-of-line-block the PE queue
                    y2_sb = res_y2.pop(rc)
                    hb = next_half()
                    for dc in range(DC):
                        nc.tensor.transpose(
                            aux_bf[hb][:, dc * NP:(dc + 1) * NP],
                            y2_sb[:, dc * NP:(dc + 1) * NP], ident_sb[:])
                    nc.vector.tensor_copy(
                        out=y2T[:, :, rc * NP:(rc + 1) * NP],
                        in_=aux_bf[hb][:].rearrange("p (d c) -> p d c", d=DC))

                # ---- group 0 runs eagerly (gates qc==0) ----
                for sc in range(4):
                    emit_ln_chunk(sc)
                for sc in range(4):
                    emit_v(sc)
                for hp in range(4):
                    emit_qk(hp, wq_sb, qT[hp], 0)
                    emit_qk(hp, wk_sb, kT[hp], 0)
                # wproj load deferred out of the startup DMA window (first
                # needed by emit_proj at qc==2)
                nc.sync.dma_start(out=wp_sb[:], in_=wproj_p[:, :, :])

                def emit_one_proj(qs, cc_dst, r):
                    attn_sb = attn_pool.tile([NP, D], BF16, tag="attnsb", name="attn_sb")
                    for nch in range(2):
                        hb = next_half()
                        av = aux[:, hb, :]
                        for pair in range(4):
                            nc.tensor.matmul(
                                av,
                                ctxT[pair][:, qs * NP:(qs + 1) * NP],
                                wp_sb[:, pair, nch * 512:(nch + 1) * 512],
                                start=(pair == 0), stop=(pair == 3))
                        nc.vector.tensor_copy(out=attn_sb[:, nch * 512:(nch + 1) * 512], in_=av)
                    nc.sync.dma_start(out=cc_dst[r * NP:(r + 1) * NP, :], in_=attn_sb[:])

                work = deque()
                pending_norm = []

                def flush_norm():
                    while pending_norm:
                        pending_norm.pop(0)()

                def make_norm(ctx_ps, hp, po, qbase):
                    def emit():
                        den = small.tile([NP, 512], F32R, tag="den", name="den")
                        nc.vector.tensor_copy(out=den[64:65, :], in_=ctx_ps[64:65, :])
                        hb = next_half()
                        b_ps = aux[0:64, hb, :]
                        nc.tensor.matmul(b_ps, ones_sb[64:65, :], den[64:65, :],
                                         start=True, stop=True)
                        b_sb = small.tile([64, 512], F32, tag="bsb", name="b_sb")
                        nc.vector.reciprocal_approx_fast(out=b_sb[:], in_=b_ps)
                        nc.vector.tensor_mul(
                            out=ctxT[hp][po:po + 64, qbase:qbase + 512],
                            in0=ctx_ps[0:64, :], in1=b_sb[:])
                    return emit

                for qc in range(QC):
                    qbase = qc * 512
                    kcs = [4 * qc] + list(range(0, 4 * qc)) + [4 * qc + 1, 4 * qc + 2, 4 * qc + 3]
                    nsteps = 4 * len(kcs)
                    qstep = 0
                    # group qc+1's prep drips into this qc's attention
                    # steps; min-step gates keep a dripped item's PE work from
                    # head-of-line-blocking the PE queue behind a producer
                    # (DMA / DVE) that hasn't caught up yet
                    if qc < 3:
                        g = qc + 1
                        for sc in range(4 * g, 4 * g + 4):
                            work.append((1, lambda sc=sc: emit_x(sc)))
                        for j, sc in enumerate(range(4 * g, 4 * g + 4)):
                            work.append((3 + j, lambda sc=sc: emit_ln_chunk(sc)))
                        for j, sc in enumerate(range(4 * g, 4 * g + 4)):
                            work.append((7 + j, lambda sc=sc: emit_v(sc)))
                        for hp in range(4):
                            work.append((9, lambda hp=hp, g=g: emit_qk(hp, wq_sb, qT[hp], g)))
                            work.append((9, lambda hp=hp, g=g: emit_qk(hp, wk_sb, kT[hp], g)))
                    for hp in range(4):
                        ctxA = ctx_ps_pool.tile([HD + 1, 512], F32, tag="ctxA", name="ctxA")
                        ctxB = ctx_ps_pool.tile([HD + 1, 512], F32, tag="ctxB", name="ctxB")
                        pend = None
                        for i, kc in enumerate(kcs):
                            off = max(0, 128 * kc - qbase)
                            s2 = sc_ps.tile([NP, 2, 512], F32, tag="sc", name="s2")
                            # concurrent in the PE array: row groups 0-63 / 64-127
                            nc.tensor.matmul(
                                s2[:, 0, off:512],
                                kT[hp][0:64, kc * NP:(kc + 1) * NP],
                                qT[hp][0:64, qbase + off:qbase + 512],
                                start=True, stop=True)
                            nc.tensor.matmul(
                                s2[:, 1, off:512],
                                kT[hp][64:128, kc * NP:(kc + 1) * NP],
                                qT[hp][64:128, qbase + off:qbase + 512],
                                start=True, stop=True)
                            e2 = e_pool.tile([NP, 2, 512], BF16, tag="esb", name="e2")
                            # ONE activation over both heads' PSUM banks:
                            # halves the per-step ACTIVATE dispatch overhead
                            nc.scalar.activation(out=e2[:, :, off:512], in_=s2[:, :, off:512], func=AF.Exp)
                            if 4 * qc <= kc:
                                nc.vector.tensor_mul(
                                    out=e2[:, 0, off:off + 128], in0=e2[:, 0, off:off + 128], in1=mask_sb[:])
                                nc.vector.tensor_mul(
                                    out=e2[:, 1, off:off + 128], in0=e2[:, 1, off:off + 128], in1=mask_sb[:])
                            qstep += 1
                            if i == 0:
                                # previous pair's softmax normalization, deferred
                                # past this pair's first scores/exp so the
                                # scalar-engine exp stream never stalls at the
                                # pair boundary (must drain before this pair's
                                # first ctx matmul reuses the ctx banks)
                                flush_norm()
                            else:
                                budget = 2 if len(work) > nsteps - qstep else 1
                                while budget and work and work[0][0] <= qstep:
                                    work.popleft()[1]()
                                    budget -= 1
                            if pend is not None:
                                off_, e2_, kc_, first_ = pend
                                nc.tensor.matmul(
                                    ctxA[:, off_:512], vaug[kc_][:, 2 * hp, :],
                                    e2_[:, 0, off_:512], start=first_, stop=False)
                                nc.tensor.matmul(
                                    ctxB[:, off_:512], vaug[kc_][:, 2 * hp + 1, :],
                                    e2_[:, 1, off_:512], start=first_, stop=False)
                            pend = (off, e2, kc, i == 0)
                        off_, e2_, kc_, first_ = pend
                        nc.tensor.matmul(
                            ctxA[:, off_:512], vaug[kc_][:, 2 * hp, :],
                            e2_[:, 0, off_:512], start=first_, stop=True)
                        nc.tensor.matmul(
                            ctxB[:, off_:512], vaug[kc_][:, 2 * hp + 1, :],
                            e2_[:, 1, off_:512], start=first_, stop=True)
                        pending_norm.append(make_norm(ctxA, hp, 0, qbase))
                        pending_norm.append(make_norm(ctxB, hp, 64, qbase))

                    while work:
                        work.popleft()[1]()
                    if qc == 1:
                        work.extend(
                            (6 + 3 * qs, lambda qs=qs: emit_one_proj(qs, cc_in0, qs))
                            for qs in range(8))
                    if qc == 2:
                        nc.gpsimd.collective_compute(
                            "ReduceScatter", OP.add,
                            replica_groups=[[0, 1], [2, 3], [4, 5], [6, 7]],
                            ins=[cc_in0[:].opt()], outs=[cc_out0[:].opt()])
                        # qs 8-11 (rows 1024-1535) only need qc<=2 context;
                        # dripped into qc==3 a few steps in, so their ctxT
                        # reads aren't gated on qc2 norms still in the DVE queue
                        work.extend(
                            (8 + 3 * (qs - 8), lambda qs=qs: emit_one_proj(qs, cc_in1, qs - 8))
                            for qs in range(8, 12))
                        # FFN half-0 prologue, gated on RS#1: DVE part drips
                        # once the collective has surely landed, PE transposes
                        # a few steps later still
                        work.extend(
                            (40 + 3 * rc, lambda rc=rc: emit_residual_dve(rc))
                            for rc in range(4))
                        work.extend(
                            (53 + 3 * rc, lambda rc=rc: emit_residual_pe(rc))
                            for rc in range(4))
                    if qc == 3:
                        flush_norm()
                        for qs in range(12, 16):
                            emit_one_proj(qs, cc_in1, qs - 8)

            # RS#2 issued OUTSIDE the attention pool scopes: the pool-stack
            # close drains all engines, so a collective issued inside would
            # serialize the whole FFN behind its completion.
            nc.gpsimd.collective_compute(
                "ReduceScatter", OP.add,
                replica_groups=[[0, 1], [2, 3], [4, 5], [6, 7]],
                ins=[cc_in1[:].opt()], outs=[cc_out1[:].opt()])

            # ---------------- FFN phase (1024 rows per core) ----------------
            with contextlib.ExitStack() as ffn:
                g_pool = ffn.enter_context(tc.tile_pool(name="g", bufs=64))
                y47_pool = ffn.enter_context(tc.tile_pool(name="y47", bufs=1))
                yw = ffn.enter_context(tc.tile_pool(name="yw", bufs=3))
                lnst2 = ffn.enter_context(tc.tile_pool(name="lnst2", bufs=4))
                w1_pool = ffn.enter_context(tc.tile_pool(name="w1", bufs=10))
                w2_pool = ffn.enter_context(tc.tile_pool(name="w2", bufs=8))
                out_pool = ffn.enter_context(tc.tile_pool(name="outp", bufs=3))
                ff_ps_pool = ffn.enter_context(tc.tile_pool(name="ff_ps", bufs=2, space="PSUM"))
                z_ps_pool = ffn.enter_context(tc.tile_pool(name="z_ps", bufs=1, space="PSUM"))
                tp_ps = ffn.enter_context(tc.tile_pool(name="tp_ffn", bufs=2, space="PSUM"))

                ys = y + [y47_pool.tile([NP, D], F32, tag=f"y{rc}", name=f"y{rc}") for rc in range(4, 8)]
                for half in range(2):
                    if half == 1:
                        # residual + LN2 for half 1 (half 0 ran inside the
                        # fused phase); DMAs on gpsimd so the RS#2 gate never
                        # blocks w1/w2 streaming on the sync queue
                        for r4 in range(4):
                            rc = 4 + r4
                            rs_sb = yw.tile([NP, D], BF16, tag="rs")
                            nc.gpsimd.dma_start(out=rs_sb[:], in_=cc_out1[r4 * NP:(r4 + 1) * NP, :])
                            xr_sb = yw.tile([NP, D], F32, tag="xr")
                            nc.gpsimd.dma_start(out=xr_sb[:], in_=xres_p[rc * NP:(rc + 1) * NP, :])
                            nc.vector.tensor_copy(out=ys[rc][:], in_=rs_sb[:])
                            nc.vector.tensor_add(out=ys[rc][:], in0=ys[rc][:], in1=xr_sb[:])
                            if add_bproj:
                                nc.vector.tensor_add(out=ys[rc][:], in0=ys[rc][:], in1=bproj_bc[:])
                            mv = _ln_stats(nc, lnst2, ys[rc][:], eps_sb)
                            y2_sb = yw.tile([NP, D], BF16, tag="y2")
                            nc.vector.tensor_scalar(
                                out=y2_sb[:], in0=ys[rc][:],
                                scalar1=mv[:, 0:1], scalar2=mv[:, 1:2],
                                op0=OP.subtract, op1=OP.mult)
                            if apply_ln2_affine:
                                nc.vector.tensor_mul(out=y2_sb[:], in0=y2_sb[:], in1=ln2w_bc[:])
                                nc.vector.tensor_add(out=y2_sb[:], in0=y2_sb[:], in1=ln2b_bc[:])
                            for dc in range(DC):
                                t_ps = tp_ps.tile([NP, NP], BF16, tag="tp")
                                nc.tensor.transpose(t_ps[:], y2_sb[:, dc * NP:(dc + 1) * NP], ident_sb[:])
                                nc.scalar.copy(out=y2T[:, dc, rc * NP:(rc + 1) * NP], in_=t_ps[:])

                    hcols = slice(half * 512, (half + 1) * 512)
                    gts = []
                    for ffc in range(FC):
                        w1_sb = w1_pool.tile([NP, DC * NP], BF16, tag="w1")
                        nc.sync.dma_start(out=w1_sb[:], in_=wff1_p[ffc, :, :])
                        ff_ps = ff_ps_pool.tile([NP, 512], F32, tag="ffps")
                        for dc in range(DC):
                            nc.tensor.matmul(
                                ff_ps[:],
                                w1_sb[:, dc * NP:(dc + 1) * NP],
                                y2T[:, dc, hcols],
                                start=(dc == 0), stop=(dc == DC - 1))
                        g_sb = g_pool.tile([NP, 512], BF16, tag="g")
                        nc.scalar.activation(out=g_sb[:], in_=ff_ps[:], func=AF.Gelu,
                                             bias=(bff1_sb[:, ffc:ffc + 1] if add_bff1 else 0.0),
                                             scale=1.0)
                        gts.append(g_sb)
                    for nch in range(2):
                        ncols = slice(nch * 512, (nch + 1) * 512)
                        z_pss = [z_ps_pool.tile([NP, 512], F32, tag=f"zps{r}", name=f"zps{r}") for r in range(4)]
                        for ffc in range(FC):
                            w2_sb = w2_pool.tile([NP, 512], BF16, tag="w2")
                            nc.sync.dma_start(out=w2_sb[:], in_=wff2_p[ffc, :, ncols])
                            for r4 in range(4):
                                nc.tensor.matmul(
                                    z_pss[r4][:],
                                    gts[ffc][:, r4 * NP:(r4 + 1) * NP],
                                    w2_sb[:],
                                    start=(ffc == 0), stop=(ffc == FC - 1))
                        for r4 in range(4):
                            rc = half * 4 + r4
                            o_sb = out_pool.tile([NP, 512], F32, tag="osb")
                            nc.vector.tensor_add(out=o_sb[:], in0=z_pss[r4][:], in1=ys[rc][:, ncols])
                            if add_bff2:
                                nc.vector.tensor_add(out=o_sb[:], in0=o_sb[:], in1=bff2_bc[:, ncols])
                            nc.sync.dma_start(out=out_p[rc * NP:(rc + 1) * NP, ncols], in_=o_sb[:])

    nc.compile()
    return nc


# ------------------------- host-side driver -------------------------

_BF = ml_dtypes.bfloat16


def _core_rows(g):
    return np.r_[512 * g:512 * g + 512, 1024 + 512 * g:1536 + 512 * g]


def _prep_core_inputs(inputs, b, g, flags):
    x = np.asarray(inputs["x"], np.float32)
    w_qkv = np.asarray(inputs["w_qkv"], np.float32).reshape(D, H, HD, 3)
    hs = slice(g * GH, (g + 1) * GH)
    w_k = w_qkv[:, hs, :, 0].reshape(D, GH * HD)
    w_q = (w_qkv[:, hs, :, 1] * (HD ** -0.5)).reshape(D, GH * HD)
    w_v = w_qkv[:, hs, :, 2].reshape(D, GH * HD)

    def tile_kxm(w):  # [D, 512] -> [128, DC, 512]
        return np.ascontiguousarray(w.reshape(DC, NP, GH * HD).transpose(1, 0, 2))

    w_proj = np.asarray(inputs["w_proj"], np.float32)
    wp = np.ascontiguousarray(
        w_proj[g * 512:(g + 1) * 512, :].reshape(4, NP, D).transpose(1, 0, 2))

    w_ff1 = np.asarray(inputs["w_ff1"], np.float32)
    w1t = np.ascontiguousarray(
        w_ff1.reshape(DC, NP, FC, NP).transpose(2, 1, 0, 3).reshape(FC, NP, DC * NP))
    w_ff2 = np.asarray(inputs["w_ff2"], np.float32)
    w2t = np.ascontiguousarray(w_ff2.reshape(FC, NP, D))

    j = np.arange(NP)[:, None]
    i = np.arange(NP)[None, :]
    mask = (j <= i).astype(np.float32)

    m = {
        "x": np.ascontiguousarray(x[b]),
        "x_res": np.ascontiguousarray(x[b][_core_rows(g)]),
        "wq": tile_kxm(w_q).astype(_BF),
        "wk": tile_kxm(w_k).astype(_BF),
        "wv": tile_kxm(w_v).astype(_BF),
        "wproj": wp.astype(_BF),
        "wff1": w1t.astype(_BF),
        "wff2": w2t.astype(_BF),
        "ident": np.eye(NP, dtype=_BF),
        "mask": mask.astype(_BF),
    }
    if flags[0]:
        m["bproj"] = np.asarray(inputs["b_proj"], np.float32).reshape(1, D).copy()
    if flags[1]:
        m["bff1"] = np.ascontiguousarray(np.asarray(inputs["b_ff1"], np.float32).reshape(FC, NP).T)
    if flags[2]:
        m["bff2"] = np.asarray(inputs["b_ff2"], np.float32).reshape(1, D).copy()
    return m


_NC_CACHE = {}


def kernel(**inputs):
    ln1w = np.asarray(inputs["ln1_w"], np.float32)
    ln1b = np.asarray(inputs["ln1_b"], np.float32)
    ln2w = np.asarray(inputs["ln2_w"], np.float32)
    ln2b = np.asarray(inputs["ln2_b"], np.float32)
    a1 = not (np.allclose(ln1w, 1.0) and np.allclose(ln1b, 0.0))
    a2 = not (np.allclose(ln2w, 1.0) and np.allclose(ln2b, 0.0))
    bp = not np.allclose(np.asarray(inputs["b_proj"], np.float32), 0.0)
    b1 = not np.allclose(np.asarray(inputs["b_ff1"], np.float32), 0.0)
    b2 = not np.allclose(np.asarray(inputs["b_ff2"], np.float32), 0.0)
    flags = (bp, b1, b2)

    key = (a1, a2, bp, b1, b2)
    if key not in _NC_CACHE:
        _NC_CACHE[key] = build(apply_ln1_affine=a1, apply_ln2_affine=a2,
                               add_bproj=bp, add_bff1=b1, add_bff2=b2)
    nc = _NC_CACHE[key]

    in_maps = []
    for core in range(8):
        b, g = core // 2, core % 2
        m = _prep_core_inputs(inputs, b, g, flags)
        if a1:
            m["ln1w"] = ln1w.reshape(1, D).copy()
            m["ln1b"] = ln1b.reshape(1, D).copy()
        if a2:
            m["ln2w"] = ln2w.reshape(1, D).copy()
            m["ln2b"] = ln2b.reshape(1, D).copy()
        in_maps.append(m)

    res = run_bass_kernel_spmd(nc, in_maps, core_ids=list(range(8)))

    out = np.empty((B, S, D), np.float32)
    for core in range(8):
        b, g = core // 2, core % 2
        out[b][_core_rows(g)] = res.results[core]["out"]
    return out
